# revision 5
# baseline (speedup 1.0000x reference)
"""Causal self-attention (B=8, T=1024, C=1024, H=16, hd=64) on 8 TRN2 cores.

Sharding: data parallel — one batch element per NeuronCore. Each core runs
q/k/v projections + RoPE + causal attention + output projection for its
batch element. All matmuls use float32r (full PE rate at N>=256, ~1.5e-4
rel err vs fp32).

Device layouts (partition dim first):
  xT        [C, T]  x[b].T; moving operand for Q/K proj, stationary for V.
  Q^T, K^T  [d, t]  head-pair hp occupies a [128, T] strip; a per-head
                    feature permutation (evens-then-odds) is folded into
                    the weights so RoPE's q1/q2 split is two contiguous
                    32-partition blocks per 64-row head.
  RoPE: qrot = (q + b) * C2 + (swap32(q) + swap32(b)) * S2m, swap32 done
        with 4 PSUM->SBUF DMAs per tile; C2/S2m are [128, T] cos/sin
        stacks shared by all head pairs.
  S^T   [s, t] per head: lhsT = Krot^T [64, 128] (stationary), rhs =
        Qrot^T [64, <=512]. Head pairs pack into PE row groups (K=64).
        Fully-masked s-blocks are skipped; diagonal blocks compute only
        their valid columns.
  P~ = exp(S^T/8) on ACT straight out of PSUM (float32r out); the
        diagonal 128x128 triangle gets a 0/1 multiply on DVE.
  y^T   [65, t] = [V_j | ones].T @ P~ accumulated over s-tiles j; row 64
        is the softmax denominator r. 1/r is partition-broadcast on
        GPSIMD and multiplied in on DVE while copying to Y^T.
  O^T   [e, t]  out projection of Y^T; host transposes back.
"""
import numpy as np
import concourse.bass as bass
import concourse.tile as tile
import concourse.mybir as mybir
from concourse import bacc
from concourse.bass_utils import run_bass_kernel_spmd

F32 = mybir.dt.float32
F32R = mybir.dt.float32r
EXP = mybir.ActivationFunctionType.Exp
IDENT = mybir.ActivationFunctionType.Identity
ADD = mybir.AluOpType.add
MULT = mybir.AluOpType.mult

B, T, C = 8, 1024, 1024
H, HD = 16, 64
NCORES = 8
TCH = T // 512


def build_program():
    nc = bacc.Bacc("TRN2", target_bir_lowering=False, debug=False)

    def din(name, shape, dt=F32R):
        return nc.dram_tensor(name, shape, dt, kind="ExternalInput").ap()

    xT = din("xT", [C, T])
    wqT = din("wqT", [C, C])
    wkT = din("wkT", [C, C])
    wvT = din("wvT", [C, C])
    woT = din("woT", [C, C])
    bq = din("bq", [128, 8], F32)
    bqs = din("bqs", [128, 8], F32)
    bk = din("bk", [128, 8], F32)
    bks = din("bks", [128, 8], F32)
    bo = din("bo", [128, 8], F32)
    bv = din("bv", [1, C])
    c2 = din("c2", [128, T], F32)
    s2m = din("s2m", [128, T], F32)
    tri = din("tri", [128, 128])
    onesrow = din("onesrow", [1, 128])
    ones16 = din("ones16", [128, 16])
    oT = nc.dram_tensor("oT", [C, T], F32, kind="ExternalOutput").ap()

    with tile.TileContext(nc) as tc:
        with (
            tc.tile_pool(name="pc", bufs=1) as pc,
            tc.tile_pool(name="pw", bufs=3) as pw,
            tc.tile_pool(name="pwv", bufs=2) as pwv,
            tc.tile_pool(name="prope", bufs=3) as prope,
            tc.tile_pool(name="ppt", bufs=6) as ppt,
            tc.tile_pool(name="pnorm", bufs=3) as pnorm,
            tc.tile_pool(name="posb", bufs=2) as posb,
            tc.tile_pool(name="psA", bufs=3, space="PSUM") as psA,
            tc.tile_pool(name="psS", bufs=3, space="PSUM") as psS,
            tc.tile_pool(name="psY", bufs=2, space="PSUM") as psY,
        ):
            # ---- resident tensors ----
            xT_sb = pc.tile([128, 8 * T], F32R, tag="xbig")
            nc.sync.dma_start(
                xT_sb[:].rearrange("p (ct t) -> p ct t", ct=8),
                xT.rearrange("(ct p) t -> p ct t", p=128),
            )
            c2_sb = pc.tile([128, T], F32, tag="c2")
            s2_sb = pc.tile([128, T], F32, tag="s2")
            nc.sync.dma_start(c2_sb[:], c2)
            nc.sync.dma_start(s2_sb[:], s2m)
            tri_sb = pc.tile([128, 128], F32R, tag="tri")
            nc.sync.dma_start(tri_sb[:], tri)
            onesrow_sb = pc.tile([1, 128], F32R, tag="onesrow")
            nc.sync.dma_start(onesrow_sb[:], onesrow)
            bv_sb = pc.tile([1, C], F32R, tag="bv")
            nc.sync.dma_start(bv_sb[:], bv)
            btiles = {}
            for nm, ap in [("bq", bq), ("bqs", bqs), ("bk", bk), ("bks", bks),
                           ("bo", bo)]:
                t_ = pc.tile([128, 8], F32, tag=nm)
                nc.sync.dma_start(t_[:], ap)
                btiles[nm] = t_
            qrot_sb = pc.tile([128, 8 * T], F32R, tag="qrot")
            krot_sb = pc.tile([128, 8 * T], F32R, tag="krot")
            # V per s-tile: [128, 16 heads x (64 cols + ones col)]
            v_sb = [pc.tile([128, 16 * 65], F32R, tag=f"v{j}", name=f"v{j}")
                    for j in range(8)]
            v3 = [v_sb[j][:].rearrange("p (h j) -> p h j", j=65) for j in range(8)]
            for j in range(8):
                nc.sync.dma_start(v3[j][:, :, 64:65], ones16)

            # ---- Q/K projections + RoPE ----
            for wT, bnm, bsnm, dest in [
                (wqT, "bq", "bqs", qrot_sb),
                (wkT, "bk", "bks", krot_sb),
            ]:
                for dblk in range(8):
                    wsl = pw.tile([128, 8, 128], F32R, tag="w")
                    nc.sync.dma_start(
                        wsl[:],
                        wT[:, dblk * 128:(dblk + 1) * 128].rearrange(
                            "(ct p) m -> p ct m", p=128),
                    )
                    for ch in range(TCH):
                        ps = psA.tile([128, 512], F32, tag="mm")
                        for ct in range(8):
                            nc.tensor.matmul(
                                ps[:],
                                wsl[:, ct, :],
                                xT_sb[:, ct * T + ch * 512: ct * T + ch * 512 + 512],
                                start=(ct == 0), stop=(ct == 7),
                            )
                        # swap q1/q2 16-row halves within each 32-partition
                        # quadrant (features are interleave-16 permuted on
                        # host, so this pairs q1[i] with q2[i])
                        qsw = prope.tile([128, 512], F32, tag="qsw")
                        nc.vector.stream_shuffle(
                            qsw[:], ps[:],
                            mask=list(range(16, 32)) + list(range(0, 16)))
                        dsl = dest[:, dblk * T + ch * 512: dblk * T + ch * 512 + 512]
                        nc.vector.scalar_tensor_tensor(
                            dsl, ps[:], btiles[bnm][:, dblk:dblk + 1],
                            c2_sb[:, ch * 512:ch * 512 + 512], op0=ADD, op1=MULT)
                        nc.vector.scalar_tensor_tensor(
                            qsw[:], qsw[:], btiles[bsnm][:, dblk:dblk + 1],
                            s2_sb[:, ch * 512:ch * 512 + 512], op0=ADD, op1=MULT)
                        nc.vector.tensor_add(dsl, dsl, qsw[:])

            # ---- V projection (ones-bias rank-1 fold + strided copy) ----
            for ch in range(TCH):
                wv_r = wvT[:, ch * 512:(ch + 1) * 512].rearrange(
                    "(ct p) m -> p ct m", p=128)
                vsl0 = pwv.tile([128, 4, 512], F32R, tag="wv")
                nc.sync.dma_start(vsl0[:], wv_r[:, 0:4, :])
                vsl1 = pwv.tile([128, 4, 512], F32R, tag="wv")
                nc.sync.dma_start(vsl1[:], wv_r[:, 4:8, :])
                for sblk in range(8):
                    ps = psA.tile([128, 512], F32, tag="mm")
                    for ct in range(8):
                        vsl = vsl0 if ct < 4 else vsl1
                        nc.tensor.matmul(
                            ps[:],
                            xT_sb[:, ct * T + sblk * 128: ct * T + sblk * 128 + 128],
                            vsl[:, ct % 4, :],
                            start=(ct == 0), stop=False,
                        )
                    nc.tensor.matmul(
                        ps[:], onesrow_sb[:], bv_sb[:, ch * 512:(ch + 1) * 512],
                        start=False, stop=True,
                    )
                    nc.scalar.copy(v3[sblk][:, 8 * ch:8 * ch + 8, 0:64], ps[:])

            # ---- attention, one head pair at a time ----
            yt_sb = pc.tile([128, 8 * T], F32R, tag="xbig")  # reuses xT's slot
            for hp in range(8):
                base = hp * T
                for c in range(TCH):
                    njs = 4 * c + 4
                    ps_y = [psY.tile([65, 512], F32, tag="y", name=f"y{hp}_{c}_{k}")
                            for k in range(2)]
                    for j in range(njs):
                        nst = 128 * (j - 4 * c) if j >= 4 * c else 0
                        p_ts = []
                        for hi in range(2):
                            r0 = 64 * hi
                            ps_s = psS.tile([128, 512], F32, tag="s")
                            nc.tensor.matmul(
                                ps_s[:, nst:512],
                                krot_sb[r0:r0 + 64, base + j * 128: base + j * 128 + 128],
                                qrot_sb[r0:r0 + 64,
                                        base + c * 512 + nst: base + c * 512 + 512],
                                start=True, stop=True,
                            )
                            p_t = ppt.tile([128, 512], F32R, tag="pt")
                            nc.scalar.activation(p_t[:, nst:512], ps_s[:, nst:512],
                                                 EXP, scale=0.125)
                            if j >= 4 * c:
                                nc.vector.tensor_mul(p_t[:, nst:nst + 128],
                                                     p_t[:, nst:nst + 128],
                                                     tri_sb[:])
                            p_ts.append(p_t)
                        for hi in range(2):
                            h = 2 * hp + hi
                            nc.tensor.matmul(
                                ps_y[hi][:, nst:512],
                                v_sb[j][:, 65 * h: 65 * h + 65],
                                p_ts[hi][:, nst:512],
                                start=(j == 0), stop=(j == njs - 1),
                            )
                    for hi in range(2):
                        rec = pnorm.tile([1, 512], F32, tag="rec")
                        nc.vector.reciprocal(rec[:], ps_y[hi][64:65, :])
                        rbc = pnorm.tile([64, 512], F32, tag="rbc")
                        nc.gpsimd.partition_broadcast(rbc[:], rec[:])
                        nc.vector.tensor_mul(
                            yt_sb[64 * hi:64 * hi + 64,
                                  base + c * 512: base + c * 512 + 512],
                            ps_y[hi][0:64, :], rbc[:])

            # ---- output projection ----
            for eblk in range(8):
                wsl = pw.tile([128, 8, 128], F32R, tag="w")
                nc.sync.dma_start(
                    wsl[:],
                    woT[:, eblk * 128:(eblk + 1) * 128].rearrange(
                        "(ct p) m -> p ct m", p=128),
                )
                for ch in range(TCH):
                    ps = psA.tile([128, 512], F32, tag="mm")
                    for dt in range(8):
                        nc.tensor.matmul(
                            ps[:],
                            wsl[:, dt, :],
                            yt_sb[:, dt * T + ch * 512: dt * T + ch * 512 + 512],
                            start=(dt == 0), stop=(dt == 7),
                        )
                    osb = posb.tile([128, 512], F32, tag="osb")
                    nc.scalar.activation(osb[:], ps[:], IDENT,
                                         bias=btiles["bo"][:, eblk:eblk + 1])
                    nc.sync.dma_start(
                        oT[eblk * 128:(eblk + 1) * 128, ch * 512:(ch + 1) * 512],
                        osb[:])

    nc.compile()
    return nc


def prep_inputs(x, wq, bq, wk, bk, wv, bv, wo, bo):
    """Host-side prep: per-head feature permutation, transposes, RoPE tables."""
    f32 = np.float32
    # interleave-16 feature order per head: [q1[0:16], q2[0:16],
    # q1[16:32], q2[16:32]] where q1 = even orig features, q2 = odd.
    perm = np.concatenate([
        np.arange(0, 32, 2), np.arange(1, 32, 2),
        np.arange(32, 64, 2), np.arange(33, 64, 2),
    ])
    pidx = np.concatenate([h * HD + perm for h in range(H)])

    wq_p, bq_p = wq[pidx], bq[pidx]
    wk_p, bk_p = wk[pidx], bk[pidx]
    # swap the 16-row halves within every 32-row quadrant
    swap = lambda v: np.ascontiguousarray(
        v.reshape(2 * H, 2, 16)[:, ::-1].reshape(-1))
    bt = lambda v: np.ascontiguousarray(v.reshape(8, 128).T, dtype=f32)

    inv_freq = (1.0 / (10000.0 ** (np.arange(0, HD, 2, dtype=np.float64) / HD)))
    th = np.outer(np.arange(T, dtype=np.float64), inv_freq)  # [T, 32]
    cosT = np.cos(th).T.astype(f32)  # [32, T]
    sinT = np.sin(th).T.astype(f32)
    c64 = np.concatenate([cosT[0:16], cosT[0:16], cosT[16:32], cosT[16:32]])
    s64 = np.concatenate([-sinT[0:16], sinT[0:16], -sinT[16:32], sinT[16:32]])
    c2 = np.ascontiguousarray(np.tile(c64, (2, 1)))  # [128, T]
    s2m = np.ascontiguousarray(np.tile(s64, (2, 1)))

    shared = {
        "wqT": np.ascontiguousarray(wq_p.T, dtype=f32),
        "wkT": np.ascontiguousarray(wk_p.T, dtype=f32),
        "wvT": np.ascontiguousarray(wv.T, dtype=f32),
        "woT": np.ascontiguousarray(wo.T, dtype=f32),
        "bq": bt(bq_p), "bqs": bt(swap(bq_p)),
        "bk": bt(bk_p), "bks": bt(swap(bk_p)),
        "bo": bt(bo),
        "bv": np.ascontiguousarray(bv[None, :], dtype=f32),
        "c2": c2, "s2m": s2m,
        "tri": np.triu(np.ones((128, 128), dtype=f32)),
        "onesrow": np.ones((1, 128), dtype=f32),
        "ones16": np.ones((128, 16), dtype=f32),
    }
    in_maps = []
    for b in range(B):
        m = dict(shared)
        m["xT"] = np.ascontiguousarray(np.asarray(x[b]).T, dtype=f32)
        in_maps.append(m)
    return in_maps


_nc_cache = None


def run(inputs, trace=False, trace_kwargs=None):
    global _nc_cache
    if _nc_cache is None:
        _nc_cache = build_program()
    in_maps = prep_inputs(
        np.asarray(inputs["x"], dtype=np.float32),
        *[np.asarray(inputs[k], dtype=np.float32)
          for k in ["wq", "bq", "wk", "bk", "wv", "bv", "wo", "bo"]])
    res = run_bass_kernel_spmd(_nc_cache, in_maps, list(range(NCORES)),
                               trace=trace, **(trace_kwargs or {}))
    out = np.stack([np.ascontiguousarray(res.results[b]["oT"].T)
                    for b in range(B)]).astype(np.float32)
    return out, res


def kernel(**inputs):
    out, _ = run(inputs, trace=False)
    return out
